# revision 10
# baseline (speedup 1.0000x reference)
"""Group-equivariant conv (folds to dense 128->128ch 3x3 conv, pad=1) on 8 trn2 cores.

Strategy: data-parallel over batch (2 images/core). The G^2-shifted group conv
is expanded on host (pure index shuffle, no FLOPs) into a dense [128,128,3,3]
weight. On device each image is laid out channel-on-partition as a zero-padded
flattened plane [128, 130*130]; the 3x3 conv is 9 PSUM-accumulated matmuls per
output chunk, where tap (dy,dx)'s rhs is just a constant-offset slice of the
flattened plane. fp32r matmul dtype = full PE rate at fp32 storage.
"""

import sys

for _p in ("/opt/trn_rl_repo",):
    if _p not in sys.path:
        sys.path.insert(0, _p)

from contextlib import ExitStack

import numpy as np

import concourse.bacc as bacc
import concourse.mybir as mybir
import concourse.tile as tile
from concourse import bass_utils as _bass_utils
from concourse.bass_utils import run_bass_kernel_spmd

# Walrus's LDWEIGHTS-dedup pass is off by default; measured no speedup (matmul
# issue rate is the limiter, LDWs hide behind it) and fp32r + standalone LDW
# has a known all-zeros hazard, so leave it off.
_ENABLE_LDW_OPT = False

_orig_run_command = _bass_utils.run_command


def _patched_run_command(argv, **kwargs):
    if _ENABLE_LDW_OPT and isinstance(argv, list):
        argv = [
            "--enable-ldw-opt=true" if a == "--enable-ldw-opt=false" else a
            for a in argv
        ]
    return _orig_run_command(argv, **kwargs)


_bass_utils.run_command = _patched_run_command

NCORES = 8
B, C, H, W = 16, 128, 128, 128
BPC = B // NCORES           # images per core
S = W + 2                   # padded row stride
XCOLS = (H + 2) * S + 4     # padded plane + tail guard for last tap reads
CH = 3                      # output rows per PSUM chunk (N = 3*130 = 390 <= 512)
NBLK = 16                   # input row-blocks per image (DMA pipelining)
# chunks-per-PSUM-group schedule: image 0 ramps up so the PE can start as soon
# as the first input rows land (taps-outer consumes a whole group's rows within
# the first tap pass); both images ramp down so the post-last-matmul tail is a
# tiny copy+DMA. 43 chunks per image.
GROUPS_IMG0 = [1, 1, 2, 2, 4, 4, 8, 8, 8, 4, 1]
GROUPS_IMGN = [8, 8, 8, 8, 8, 2, 1]

F32 = mybir.dt.float32
F32R = mybir.dt.float32r


def _expand_weight(weight: np.ndarray) -> np.ndarray:
    """[32,32,4,3,3] -> lhsT layout [ci=128, tap=9, co=128] flattened [128, 1152]."""
    o, i, g, kh, kw = weight.shape
    gi = np.arange(g)
    shift = (gi[:, None] - gi[None, :]) % g            # [g, h]
    wb = weight[:, :, shift]                           # [o, i, g, h, kh, kw]
    wb = np.transpose(wb, (2, 0, 1, 3, 4, 5))          # [g, o, i, h, kh, kw]
    wb = wb.reshape(g * o, i * g, kh, kw)              # [co=128, ci=128, 3, 3]
    wt = np.transpose(wb, (1, 2, 3, 0))                # [ci, kh, kw, co]
    return np.ascontiguousarray(wt.reshape(C, 9 * C)).astype(np.float32)


def _chunks():
    out = []
    y = 0
    while y < H:
        rows = min(CH, H - y)
        out.append((y, rows))
        y += rows
    return out


def _build_body(ctx: ExitStack, tc: tile.TileContext, x_ap, wt_ap, out_ap):
    nc = tc.nc
    xpool = ctx.enter_context(tc.tile_pool(name="xp", bufs=1))
    wpool = ctx.enter_context(tc.tile_pool(name="wp", bufs=1))
    opool = ctx.enter_context(tc.tile_pool(name="op", bufs=2))
    ppool = ctx.enter_context(tc.tile_pool(name="pp", bufs=8, space="PSUM"))

    wt = wpool.tile([C, 9 * C], F32R, name="wt_sb")
    # Per-tap DMAs: the first matmul only gates on tap 0's 64KB, not the
    # whole 590KB table.
    for t in range(9):
        nc.sync.dma_start(out=wt[:, t * C:(t + 1) * C], in_=wt_ap[:, t * C:(t + 1) * C])

    xbufs = []
    for i in range(BPC):
        xb = xpool.tile([C, XCOLS], F32R, name=f"xb{i}", tag=f"xb{i}")
        xbufs.append(xb)
        # Zero only the pad cells once; interior DMAs never touch them.
        # (memset can't encode float32r — bitcast the APs to plain f32.)
        nc.vector.memset(xb[:, 0:S].bitcast(F32), 0.0)            # top pad row
        nc.vector.memset(xb[:, (H + 1) * S:XCOLS].bitcast(F32), 0.0)  # bottom row + guard
        pairs = xb[:, S - 1:S - 1 + (H + 1) * S].rearrange(
            "p (r s) -> p r s", s=S)[:, :, 0:2]                   # col pads (row ends)
        nc.vector.memset(pairs.bitcast(F32), 0.0)

    chunks = _chunks()
    RB = H // NBLK

    for img in range(BPC):
        sched = GROUPS_IMG0 if img == 0 else GROUPS_IMGN
        assert sum(sched) == len(chunks)
        groups = []
        i = 0
        for gs in sched:
            groups.append(chunks[i:i + gs])
            i += gs
        xb = xbufs[img]
        xview = xb[:, 0:(H + 2) * S].rearrange("p (r s) -> p r s", s=S)
        for blk in range(NBLK):
            r0 = blk * RB
            nc.sync.dma_start(
                out=xview[:, 1 + r0:1 + r0 + RB, 1:1 + W],
                in_=x_ap[img, :, r0:r0 + RB, :],
            )

        for grp in groups:
            g_y0 = grp[0][0]
            g_rows = sum(r for _, r in grp)
            psums = [ppool.tile([C, 512], F32, name="ps", tag="ps") for _ in grp]
            for t in range(9):
                dy, dx = divmod(t, 3)
                wslice = wt[:, t * C:(t + 1) * C]
                for pt, (y, rows) in zip(psums, grp):
                    n = rows * S
                    off = (y + dy) * S + dx
                    nc.tensor.matmul(
                        pt[:, 0:n], wslice, xb[:, off:off + n],
                        start=(t == 0), stop=(t == 8),
                    )
            stage = opool.tile([C, g_rows * W], F32, name="stage", tag="stage")
            col = 0
            for pt, (y, rows) in zip(psums, grp):
                src = pt[:, 0:rows * S].rearrange("p (r s) -> p r s", s=S)[:, :, 0:W]
                dst = stage[:, col:col + rows * W].rearrange("p (r s) -> p r s", s=W)
                nc.vector.tensor_copy(dst, src)
                col += rows * W
            # Stores go on the ACT HWDGE ring so they never queue behind the
            # (large) input loads on the SP ring.
            nc.scalar.dma_start(
                out=out_ap[img, :, g_y0:g_y0 + g_rows, :],
                in_=stage[:, 0:g_rows * W],
            )


_NC_CACHE = None


def _get_nc():
    global _NC_CACHE
    if _NC_CACHE is None:
        nc = bacc.Bacc("TRN2", target_bir_lowering=False, debug=False)
        x_ap = nc.dram_tensor("x", [BPC, C, H, W], F32R, kind="ExternalInput").ap()
        wt_ap = nc.dram_tensor("wt", [C, 9 * C], F32R, kind="ExternalInput").ap()
        out_ap = nc.dram_tensor("out", [BPC, C, H, W], F32, kind="ExternalOutput").ap()
        with tile.TileContext(nc) as tc:
            with ExitStack() as ctx:
                _build_body(ctx, tc, x_ap, wt_ap, out_ap)
        nc.compile()
        _NC_CACHE = nc
    return _NC_CACHE


def _run(x: np.ndarray, weight: np.ndarray, trace: bool = False, **kw):
    x = np.ascontiguousarray(np.asarray(x, dtype=np.float32))
    wt = _expand_weight(np.asarray(weight, dtype=np.float32))
    nc = _get_nc()
    in_maps = [
        {"x": x[c * BPC:(c + 1) * BPC], "wt": wt} for c in range(NCORES)
    ]
    res = run_bass_kernel_spmd(nc, in_maps, list(range(NCORES)), trace=trace, **kw)
    out = np.concatenate([res.results[c]["out"] for c in range(NCORES)], axis=0)
    return out, res


def kernel(x: np.ndarray, weight: np.ndarray) -> np.ndarray:
    out, _ = _run(x, weight)
    return out


# revision 15
# speedup vs baseline: 1.1140x; 1.1140x over previous
"""Group-equivariant conv (folds to dense 128->128ch 3x3 conv, pad=1) on 8 trn2 cores.

Strategy: data-parallel over batch (2 images/core). The G^2-shifted group conv
is expanded on host (pure index shuffle, no FLOPs) into a dense [128,128,3,3]
weight. On device each image is laid out channel-on-partition as a zero-padded
flattened plane [128, 130*130]; the 3x3 conv is 9 PSUM-accumulated matmuls per
output chunk, where tap (dy,dx)'s rhs is just a constant-offset slice of the
flattened plane. fp32r matmul dtype = full PE rate at fp32 storage.
"""

import sys

for _p in ("/opt/trn_rl_repo",):
    if _p not in sys.path:
        sys.path.insert(0, _p)

from contextlib import ExitStack

import numpy as np

import concourse.bacc as bacc
import concourse.mybir as mybir
import concourse.tile as tile
from concourse import bass_utils as _bass_utils
from concourse.bass_utils import run_bass_kernel_spmd

# Walrus's LDWEIGHTS-dedup pass is off by default; measured no speedup (matmul
# issue rate is the limiter, LDWs hide behind it) and fp32r + standalone LDW
# has a known all-zeros hazard, so leave it off.
_ENABLE_LDW_OPT = False

_orig_run_command = _bass_utils.run_command


def _patched_run_command(argv, **kwargs):
    if _ENABLE_LDW_OPT and isinstance(argv, list):
        argv = [
            "--enable-ldw-opt=true" if a == "--enable-ldw-opt=false" else a
            for a in argv
        ]
    return _orig_run_command(argv, **kwargs)


_bass_utils.run_command = _patched_run_command

NCORES = 8
B, C, H, W = 16, 128, 128, 128
BPC = B // NCORES           # images per core
S = W + 2                   # padded row stride
XCOLS = (H + 2) * S + 4     # padded plane + tail guard for last tap reads
CH = 3                      # output rows per PSUM chunk (N = 3*130 = 390 <= 512)
# input row-block schedule per image: image 0 front-loads small blocks so the
# first matmul's gate (block 0 completion) clears ASAP.
BLOCKS_IMG0 = [4, 4] + [8] * 15
BLOCKS_IMGN = [8] * 16
# chunks-per-PSUM-group schedule: image 0 ramps up so the PE can start as soon
# as the first input rows land (taps-outer consumes a whole group's rows within
# the first tap pass); both images ramp down so the post-last-matmul tail is a
# tiny copy+DMA. 43 chunks per image.
GROUPS_IMG0 = [1, 1, 2, 2, 4, 4, 8, 8, 8, 4, 1]
GROUPS_IMGN = [8, 8, 8, 8, 8, 2, 1]

F32 = mybir.dt.float32
F32R = mybir.dt.float32r
BF16 = mybir.dt.bfloat16

# Moving-operand dtype for the matmuls. fp32r: exact fp32 storage, ~183ns/MM
# measured. bf16: casts inputs on load (SWDGE inline cast), ~1 cyc/col stream
# rate, ~10x larger rounding error (still ~1e-3 rel-to-scale).
MM_BF16 = True


def _expand_weight(weight: np.ndarray) -> np.ndarray:
    """[32,32,4,3,3] -> lhsT layout [ci=128, tap=9, co=128] flattened [128, 1152]."""
    o, i, g, kh, kw = weight.shape
    gi = np.arange(g)
    shift = (gi[:, None] - gi[None, :]) % g            # [g, h]
    wb = weight[:, :, shift]                           # [o, i, g, h, kh, kw]
    wb = np.transpose(wb, (2, 0, 1, 3, 4, 5))          # [g, o, i, h, kh, kw]
    wb = wb.reshape(g * o, i * g, kh, kw)              # [co=128, ci=128, 3, 3]
    wt = np.transpose(wb, (1, 2, 3, 0))                # [ci, kh, kw, co]
    return np.ascontiguousarray(wt.reshape(C, 9 * C)).astype(np.float32)


def _chunks():
    out = []
    y = 0
    while y < H:
        rows = min(CH, H - y)
        out.append((y, rows))
        y += rows
    return out


def _build_body(ctx: ExitStack, tc: tile.TileContext, x_ap, wt_ap, out_ap):
    nc = tc.nc
    mmdt = BF16 if MM_BF16 else F32R
    xpool = ctx.enter_context(tc.tile_pool(name="xp", bufs=1))
    wpool = ctx.enter_context(tc.tile_pool(name="wp", bufs=1))
    opool = ctx.enter_context(tc.tile_pool(name="op", bufs=3))
    ppool = ctx.enter_context(tc.tile_pool(name="pp", bufs=8, space="PSUM"))

    wt = wpool.tile([C, 9 * C], mmdt, name="wt_sb")
    # ACT ring (keeps the SP ring free so input block 0 starts immediately;
    # HWDGE rings are FIFO per issuing engine). Tap 0 goes first on its own so
    # the first matmul's weight gate clears after 64KB, not 590KB.
    nc.scalar.dma_start(out=wt[:, 0:C], in_=wt_ap[:, 0:C])
    nc.scalar.dma_start(out=wt[:, C:9 * C], in_=wt_ap[:, C:9 * C])

    xbufs = []
    for i in range(BPC):
        xb = xpool.tile([C, XCOLS], mmdt, name=f"xb{i}", tag=f"xb{i}")
        xbufs.append(xb)
        # Zero only the pad cells once; interior DMAs never touch them.
        # (memset can't encode float32r — bitcast those APs to plain f32.)
        cast = (lambda ap: ap) if MM_BF16 else (lambda ap: ap.bitcast(F32))
        nc.vector.memset(cast(xb[:, 0:S]), 0.0)                   # top pad row
        nc.vector.memset(cast(xb[:, (H + 1) * S:XCOLS]), 0.0)     # bottom row + guard
        pairs = xb[:, S - 1:S - 1 + (H + 1) * S].rearrange(
            "p (r s) -> p r s", s=S)[:, :, 0:2]                   # col pads (row ends)
        nc.vector.memset(cast(pairs), 0.0)

    chunks = _chunks()

    for img in range(BPC):
        sched = GROUPS_IMG0 if img == 0 else GROUPS_IMGN
        assert sum(sched) == len(chunks)
        groups = []
        i = 0
        for gs in sched:
            groups.append(chunks[i:i + gs])
            i += gs
        xb = xbufs[img]
        xview = xb[:, 0:(H + 2) * S].rearrange("p (r s) -> p r s", s=S)
        r0 = 0
        for rb in (BLOCKS_IMG0 if img == 0 else BLOCKS_IMGN):
            dst = xview[:, 1 + r0:1 + r0 + rb, 1:1 + W]
            src = x_ap[img, :, r0:r0 + rb, :]
            if MM_BF16:
                # SWDGE casts f32 -> bf16 inline during the transfer.
                nc.gpsimd.dma_start(out=dst, in_=src)
            else:
                nc.sync.dma_start(out=dst, in_=src)
            r0 += rb

        for grp in groups:
            g_y0 = grp[0][0]
            g_rows = sum(r for _, r in grp)
            psums = [ppool.tile([C, 512], F32, name="ps", tag="ps") for _ in grp]
            for t in range(9):
                dy, dx = divmod(t, 3)
                wslice = wt[:, t * C:(t + 1) * C]
                for pt, (y, rows) in zip(psums, grp):
                    n = rows * S
                    off = (y + dy) * S + dx
                    nc.tensor.matmul(
                        pt[:, 0:n], wslice, xb[:, off:off + n],
                        start=(t == 0), stop=(t == 8),
                    )
            stage = opool.tile([C, g_rows * W], F32, name="stage", tag="stage")
            col = 0
            for pt, (y, rows) in zip(psums, grp):
                src = pt[:, 0:rows * S].rearrange("p (r s) -> p r s", s=S)[:, :, 0:W]
                dst = stage[:, col:col + rows * W].rearrange("p (r s) -> p r s", s=W)
                nc.vector.tensor_copy(dst, src)
                col += rows * W
            # Stores go on the ACT HWDGE ring so they never queue behind the
            # (large) input loads on the SP ring.
            nc.scalar.dma_start(
                out=out_ap[img, :, g_y0:g_y0 + g_rows, :],
                in_=stage[:, 0:g_rows * W],
            )


_NC_CACHE = None


def _get_nc():
    global _NC_CACHE
    if _NC_CACHE is None:
        nc = bacc.Bacc("TRN2", target_bir_lowering=False, debug=False)
        xdt = F32 if MM_BF16 else F32R
        wdt = BF16 if MM_BF16 else F32R
        x_ap = nc.dram_tensor("x", [BPC, C, H, W], xdt, kind="ExternalInput").ap()
        wt_ap = nc.dram_tensor("wt", [C, 9 * C], wdt, kind="ExternalInput").ap()
        out_ap = nc.dram_tensor("out", [BPC, C, H, W], F32, kind="ExternalOutput").ap()
        with tile.TileContext(nc) as tc:
            with ExitStack() as ctx:
                _build_body(ctx, tc, x_ap, wt_ap, out_ap)
        nc.compile()
        _NC_CACHE = nc
    return _NC_CACHE


def _run(x: np.ndarray, weight: np.ndarray, trace: bool = False, **kw):
    x = np.ascontiguousarray(np.asarray(x, dtype=np.float32))
    wt = _expand_weight(np.asarray(weight, dtype=np.float32))
    if MM_BF16:
        import ml_dtypes
        wt = wt.astype(ml_dtypes.bfloat16)
    nc = _get_nc()
    in_maps = [
        {"x": x[c * BPC:(c + 1) * BPC], "wt": wt} for c in range(NCORES)
    ]
    res = run_bass_kernel_spmd(nc, in_maps, list(range(NCORES)), trace=trace, **kw)
    out = np.concatenate([res.results[c]["out"] for c in range(NCORES)], axis=0)
    return out, res


def kernel(x: np.ndarray, weight: np.ndarray) -> np.ndarray:
    out, _ = _run(x, weight)
    return out
